# revision 2
# baseline (speedup 1.0000x reference)
"""SEIR Euler integration kernel for 8 TRN2 NeuronCores — v3.

Shards the batch axis (B=32768) across 8 cores (4096 each); every core runs
the full 1024-step Euler scan on its shard and streams the trajectory to DRAM.

v3: only THREE state chains (s, e, i) live on the device — R is a pure sink
(R_{n+1} = R_n + (g/2) I_n, nothing feeds back), so the host reconstructs it
as R_0 + (g/2)·cumsum(I).  With state scaled by c1=beta/2, c2=sigma/2
(s = c1c2·S, e = c1c2·E, i = c1·I) the per-step update is 4 DVE ops, all
CONTIGUOUS [128,32] slices:
  i'  = (a3*i) + e      [stt]  (a3 = 1-gamma/2)
  D   = s * i           [tt]   (scratch, written to previous row's D slot)
  e'  = (a2*e) + D      [stt]  (a2 = 1-sigma/2)
  s'  = s - D           [tt]
Host recovers S=s/(c1c2), E=e/(c1c2), I=i/c1, R=cumsum (free rescale).

Per-core SBUF staging layout (c-outer): one fp32 tile [128, 4*K*32] per
K-step block, column = c*(K*32) + k*32 + g with slots c=0:D 1:s 2:e 3:i,
batch element b = p*32 + g.  Slots 1..3 of the whole block form ONE
contiguous run -> single full-bandwidth DMA per block into DRAM
out[128, NBLK, 3*K*32]; host unscrambles.  The D slot is never DMA'd.

From v2 (measured): all-DVE beats GPSIMD offload (Pool ops + shared SBUF
port contention); strided-AP fusion of two slots into one op is SLOWER
(strided APs force a slow DVE path); staging bufs absorb HBM DMA completion
latency.

Toolchain constraint: this container's walrus build rejects instructions
carrying >2 semaphore waits.  Sync is legalized after build (see
_legalize_sync).
"""

import sys

sys.path.insert(0, "/opt/trn_rl_repo")

import numpy as np

import concourse.bass as bass
import concourse.tile as tile
import concourse.tile_sem_assignment as _tsa
from concourse import mybir
from concourse.bass_utils import run_bass_kernel_spmd

_tsa.NUM_HWDGE_SEMS = 1
_tsa.NUM_SWDGE_GLOBAL_SEMS = 1

T = 1024
B = 32768
NCORES = 8
BS = B // NCORES  # 4096 batch elements per core
P = 128  # SBUF partitions
G = BS // P  # 32 batch elements per partition
C = 4  # slots: D, s, e, i
K = 64  # steps per DMA block
NBLK = T // K
SEG = K * G  # columns per slot per block

TRACE = False

f32 = mybir.dt.float32
mult = mybir.AluOpType.mult
add = mybir.AluOpType.add
subtract = mybir.AluOpType.subtract


def _build(t_total=T, chain=False, passes=1, tiny_out=False,
           k_steps=K, bufs=4, scratch_out=False,
           dma_off=False, compute_off=False):
    # passes>1 re-runs the whole integration (state carried over, same out
    # rows rewritten) inside one NEFF: per-call I/O is identical, so the
    # passes-differential isolates the true per-pass device time.
    # tiny_out=True (timing only) writes every block to out row 0 so the
    # per-call PJRT output copy shrinks from 48MB to <1MB per core.
    kk = k_steps
    seg = kk * G
    nblk = t_total // kk
    out_blks = 1 if tiny_out else (T // kk)
    nc = bass.Bass(trn_type="TRN2")
    init = nc.dram_tensor("initial", [4, BS], f32, kind="ExternalInput")
    beta = nc.dram_tensor("beta", [1], f32, kind="ExternalInput")
    gamma = nc.dram_tensor("gamma", [1], f32, kind="ExternalInput")
    sigma = nc.dram_tensor("sigma", [1], f32, kind="ExternalInput")
    out = nc.dram_tensor(
        "out", [P, out_blks, 3 * seg], f32, kind="ExternalOutput"
    )
    if scratch_out:
        # timing mode: full-size Internal scratch gets the real block DMAs
        # (distinct HBM addresses), while the tiny ExternalOutput keeps the
        # per-call PJRT I/O small.
        outd = nc.dram_tensor(
            "outd", [P, T // kk, 3 * seg], f32, kind="Internal"
        )
    chain_in = chain_out = None
    if chain:
        chain_in = nc.dram_tensor("chain", [1, 1], f32, kind="ExternalInput")
        chain_out = nc.dram_tensor("chain_out", [1, 1], f32, kind="ExternalOutput")

    with tile.TileContext(nc) as tc:
        with (
            tc.tile_pool(name="consts", bufs=1) as consts,
            tc.tile_pool(name="stage", bufs=bufs) as stagep,
        ):
            # ---- broadcast the three rate scalars to all partitions ----
            bt = consts.tile([P, 1], f32, tag="bt")
            gt = consts.tile([P, 1], f32, tag="gt")
            st = consts.tile([P, 1], f32, tag="st")
            for dst, src in ((bt, beta), (gt, gamma), (st, sigma)):
                src_ap = src[:]
                bcast = bass.AP(
                    tensor=src_ap.tensor,
                    offset=src_ap.offset,
                    ap=[[0, P], [1, 1]],
                )
                nc.sync.dma_start(out=dst[:, :], in_=bcast)

            # derived per-partition scalars (all prepared on DVE)
            k1t = consts.tile([P, 1], f32, tag="k1")  # c1 = beta/2
            a2t = consts.tile([P, 1], f32, tag="a2")  # 1 - sigma/2
            a3t = consts.tile([P, 1], f32, tag="a3")  # 1 - gamma/2
            c2t = consts.tile([P, 1], f32, tag="c2")  # sigma/2
            m1t = consts.tile([P, 1], f32, tag="m1")  # c1*c2
            nc.vector.tensor_scalar_mul(k1t[:, :], bt[:, :], 0.5)
            nc.vector.tensor_scalar(a2t[:, :], st[:, :], -0.5, 1.0, mult, add)
            nc.vector.tensor_scalar(a3t[:, :], gt[:, :], -0.5, 1.0, mult, add)
            nc.vector.tensor_scalar_mul(c2t[:, :], st[:, :], 0.5)
            nc.vector.tensor_mul(m1t[:, :], k1t[:, :], c2t[:, :])
            a2 = a2t[:, 0:1]
            a3 = a3t[:, 0:1]

            # ---- initial state into block 0, step row 0 ----
            tmp0 = consts.tile([P, 4 * G], f32, tag="init_tmp")
            nc.sync.dma_start(
                out=tmp0[:, :].rearrange("p (c g) -> p c g", c=4),
                in_=init[:, :].rearrange("c (p g) -> p c g", p=P),
            )
            tv = tmp0[:, :].rearrange("p (c g) -> p c g", c=4)  # S,E,I,R

            cur = stagep.tile([P, C * seg], f32, tag="stage")
            r = cur[:, :].rearrange("p (c k g) -> p c k g", c=C, k=kk)
            # row 0: s = c1c2*S ; e = c1c2*E ; i = c1*I
            nc.vector.tensor_scalar_mul(r[:, 1, 0, :], tv[:, 0, :], m1t[:, 0:1])
            nc.vector.tensor_scalar_mul(r[:, 2, 0, :], tv[:, 1, :], m1t[:, 0:1])
            nc.vector.tensor_scalar_mul(r[:, 3, 0, :], tv[:, 2, :], k1t[:, 0:1])

            prev_r, prev_k = r, 0
            first = True
            for blk in range(nblk * passes):
                if scratch_out:
                    dma_dst, blk_out = outd, blk % nblk
                else:
                    dma_dst, blk_out = out, (blk % nblk) % out_blks
                if not first and not compute_off:
                    cur = stagep.tile([P, C * seg], f32, tag="stage")
                    r = cur[:, :].rearrange("p (c k g) -> p c k g", c=C, k=kk)
                ks = range(1, kk) if first else range(kk)
                first = False
                for k in ks:
                    if compute_off:
                        break
                    pS = prev_r[:, 1, prev_k, :]
                    pE = prev_r[:, 2, prev_k, :]
                    pI = prev_r[:, 3, prev_k, :]
                    pD = prev_r[:, 0, prev_k, :]
                    # i' = (a3*i) + e
                    nc.vector.scalar_tensor_tensor(
                        r[:, 3, k, :], pI, a3, pE, mult, add
                    )
                    # D = s*i  -> PREVIOUS row's D slot
                    nc.vector.tensor_mul(pD, pS, pI)
                    # e' = (a2*e) + D
                    nc.vector.scalar_tensor_tensor(
                        r[:, 2, k, :], pE, a2, pD, mult, add
                    )
                    # s' = s - D
                    nc.vector.tensor_sub(r[:, 1, k, :], pS, pD)
                    prev_r, prev_k = r, k
                # store slots 1..3 (s,e,i) of the whole block: ONE contiguous
                # run, single DMA carrying a single DVE data wait.
                if dma_off:
                    continue
                nc.sync.dma_start(
                    out=dma_dst[:, blk_out, 0 : 3 * seg],
                    in_=cur[:, seg : 4 * seg],
                )

            if chain:
                cht = consts.tile([1, 1], f32, tag="chain")
                nc.sync.dma_start(out=cht[:, :], in_=chain_in[:, :])
                chv = consts.tile([1, 1], f32, tag="chainv")
                last_elem = r[0:1, 3, kk - 1, 0:1]
                nc.vector.tensor_scalar_mul(chv[:, :], last_elem, cht[0:1, 0:1])
                nc.sync.dma_start(out=chain_out[:, :], in_=chv[:, :])

    _legalize_sync(nc)
    return nc


def _legalize_sync(nc):
    # Legalize for walrus' sync-wait limits (2 per compute instruction, 1 per
    # DMA/Pool instruction):
    #  - DMACopy: drop DMA-lane ordering waits (FIFO ring + fixed +16 incs
    #    make them redundant when a data wait is present).
    #  - Drain: keep only the last DMA wait (the final block-store DMA waits
    #    on the compute engine, so DMA completion implies all engines).
    #  - Other instructions: drop sem-ge waits on their OWN engine's sem.
    for bb in nc.m.functions[0].blocks:
        for ins in bb.instructions:
            si = ins.sync_info
            if si is None:
                continue
            ow = si.on_wait
            if not ow or len(ow) < 2:
                continue
            kind = ins.__class__.__name__
            eng = str(ins.engine).rsplit(".", 1)[-1]
            if kind == "InstDMACopy":
                new_w = [
                    w
                    for w in ow
                    if not (
                        w.ant_name.startswith("DMAHW")
                        or w.ant_name.startswith("DMASW")
                    )
                ]
            elif kind == "InstDrain":
                dma_w = [w for w in ow if w.ant_name.startswith("DMA")]
                new_w = dma_w[-1:] if dma_w else ow[-1:]
            else:
                new_w = [
                    w
                    for w in ow
                    if not (
                        w.wait_mode == "sem-ge-imm"
                        and w.ant_name.split("_")[0] == eng
                    )
                ]
            if len(new_w) < len(ow):
                si.on_wait = new_w
                ins.sync_info = si


_nc = None


def kernel(initial, beta, gamma, sigma, t):
    global _nc
    assert int(t) == T
    initial = np.ascontiguousarray(np.asarray(initial, dtype=np.float32))
    beta = np.asarray(beta, dtype=np.float32).reshape(1)
    gamma = np.asarray(gamma, dtype=np.float32).reshape(1)
    sigma = np.asarray(sigma, dtype=np.float32).reshape(1)
    assert initial.shape == (4, B)

    if _nc is None:
        _nc = _build()

    in_maps = []
    for i in range(NCORES):
        shard = np.ascontiguousarray(initial[:, i * BS : (i + 1) * BS])
        in_maps.append(
            {"initial": shard, "beta": beta, "gamma": gamma, "sigma": sigma}
        )

    res = run_bass_kernel_spmd(
        _nc, in_maps, core_ids=list(range(NCORES)), trace=TRACE
    )
    if TRACE and res.exec_time_ns is not None:
        print(f"HW exec time: {res.exec_time_ns} ns")

    # host-side unscramble + unscale; R reconstructed as a cumsum of I
    c1 = np.float64(beta[0]) / 2.0
    c2 = np.float64(sigma[0]) / 2.0
    inv_m1 = np.float32(1.0 / (c1 * c2))
    inv_k1 = np.float32(1.0 / c1)
    half_g = np.float32(np.float64(gamma[0]) / 2.0)

    full = np.empty((T, B, 4), dtype=np.float32)
    for i in range(NCORES):
        # out [P, NBLK, 3*K*G]: slot-major ((s,e,i), k, g) per block
        arr = res.results[i]["out"].reshape(P, NBLK, 3, K, G)
        # -> (T=blk*K+k, b_local=p*G+g, slot)
        arr = arr.transpose(1, 3, 0, 4, 2).reshape(T, BS, 3)
        dst = full[:, i * BS : (i + 1) * BS, :]
        dst[:, :, 0] = arr[:, :, 0] * inv_m1  # S = s/(c1*c2)
        dst[:, :, 1] = arr[:, :, 1] * inv_m1  # E = e/(c1*c2)
        dst[:, :, 2] = arr[:, :, 2] * inv_k1  # I = i/c1
        # R_n = R_0 + (g/2)*sum_{m<n} I_m  (exclusive prefix sum over time)
        dr = np.cumsum(half_g * dst[:, :, 2], axis=0, dtype=np.float32)
        dst[0, :, 3] = initial[3, i * BS : (i + 1) * BS]
        dst[1:, :, 3] = dst[0, :, 3][None, :] + dr[:-1]
    return full.reshape(T * B, 4)


if __name__ == "__main__":
    rng = np.random.default_rng(0)
    ini = rng.random((4, B), dtype=np.float32)
    be, ga, si = (rng.random(1, dtype=np.float32) for _ in range(3))
    outv = kernel(ini, be, ga, si, T)
    print("ran, out shape", outv.shape, outv[:4])


# revision 7
# speedup vs baseline: 1.2768x; 1.2768x over previous
"""SEIR Euler integration kernel for 8 TRN2 NeuronCores — v3.

Shards the batch axis (B=32768) across 8 cores (4096 each); every core runs
the full 1024-step Euler scan on its shard and streams the trajectory to DRAM.

v4: only THREE state chains (s, e, q~=-i) live on the device — R is a pure
sink (R_{n+1} = R_n + (g/2) I_n, nothing feeds back), so the host
reconstructs it as R_0 + (g/2)·cumsum(I).  With state scaled by c1=beta/2,
c2=sigma/2 (s = c1c2·S, e = c1c2·E, q~ = -c1·I) the per-step update is
4 DVE ops, all CONTIGUOUS [128,32] slices, SCHEDULED so every operand is
>=2 instructions away from its producer (v3 measured a ~687ns/step wall
independent of op count — RAW write->read turnaround stalls, since the
4-op and 5-op step formulations timed identically):
  op1  s[k] = (q~[k-1] + 1) * s[k-1]   [stt add,mult]  (s' = s*(1-i))
  op2  q~[k] = a3*q~[k-1] - e[k-1]     [stt mult,sub]  (a3 = 1-gamma/2)
  op3  e[k] = a2*e[k-1] - N[k-1]       [stt mult,sub]  (a2 = 1-sigma/2)
  op4  N[k] = s[k] * q~[k]             [tt mult]       (N = -s*i = -D)
op4's result is consumed only by the NEXT iteration's op3; op4 reads op1
(distance 3) and op2 (distance 2); everything else is distance >=3.
Host recovers S=s/(c1c2), E=e/(c1c2), I=-q~/c1, R=cumsum (free rescale).

Per-core SBUF staging layout (c-outer): one fp32 tile [128, 4*K*32] per
K-step block, column = c*(K*32) + k*32 + g with slots c=0:N 1:s 2:q~ 3:e,
batch element b = p*32 + g.  Slots 1..3 of the whole block form ONE
contiguous run -> single full-bandwidth DMA per block into DRAM
out[128, NBLK, 3*K*32]; host unscrambles.  The N slot is never DMA'd.

From v2 (measured): all-DVE beats GPSIMD offload (Pool ops + shared SBUF
port contention); strided-AP fusion of two slots into one op is SLOWER
(strided APs force a slow DVE path); staging bufs absorb HBM DMA completion
latency.

Toolchain constraint: this container's walrus build rejects instructions
carrying >2 semaphore waits.  Sync is legalized after build (see
_legalize_sync).
"""

import sys

sys.path.insert(0, "/opt/trn_rl_repo")

import numpy as np

import concourse.bass as bass
import concourse.tile as tile
import concourse.tile_sem_assignment as _tsa
from concourse import mybir
from concourse.bass_utils import run_bass_kernel_spmd

_tsa.NUM_HWDGE_SEMS = 1
_tsa.NUM_SWDGE_GLOBAL_SEMS = 1

T = 1024
B = 32768
NCORES = 8
BS = B // NCORES  # 4096 batch elements per core
P = 128  # SBUF partitions
G = BS // P  # 32 batch elements per partition
C = 4  # slots: N, s, q~, e
K = 64  # steps per DMA block
NBLK = T // K
SEG = K * G  # columns per slot per block

TRACE = False

f32 = mybir.dt.float32
mult = mybir.AluOpType.mult
add = mybir.AluOpType.add
subtract = mybir.AluOpType.subtract


def _build(t_total=T, chain=False, passes=1, tiny_out=False,
           k_steps=K, bufs=4, scratch_out=False,
           dma_off=False, compute_off=False):
    # passes>1 re-runs the whole integration (state carried over, same out
    # rows rewritten) inside one NEFF: per-call I/O is identical, so the
    # passes-differential isolates the true per-pass device time.
    # tiny_out=True (timing only) writes every block to out row 0 so the
    # per-call PJRT output copy shrinks from 48MB to <1MB per core.
    kk = k_steps
    seg = kk * G
    nblk = t_total // kk
    out_blks = 1 if tiny_out else (T // kk)
    nc = bass.Bass(trn_type="TRN2")
    init = nc.dram_tensor("initial", [4, BS], f32, kind="ExternalInput")
    beta = nc.dram_tensor("beta", [1], f32, kind="ExternalInput")
    gamma = nc.dram_tensor("gamma", [1], f32, kind="ExternalInput")
    sigma = nc.dram_tensor("sigma", [1], f32, kind="ExternalInput")
    out = nc.dram_tensor(
        "out", [P, out_blks, 3 * seg], f32, kind="ExternalOutput"
    )
    if scratch_out:
        # timing mode: full-size Internal scratch gets the real block DMAs
        # (distinct HBM addresses), while the tiny ExternalOutput keeps the
        # per-call PJRT I/O small.
        outd = nc.dram_tensor(
            "outd", [P, T // kk, 3 * seg], f32, kind="Internal"
        )
    chain_in = chain_out = None
    if chain:
        chain_in = nc.dram_tensor("chain", [1, 1], f32, kind="ExternalInput")
        chain_out = nc.dram_tensor("chain_out", [1, 1], f32, kind="ExternalOutput")

    with tile.TileContext(nc) as tc:
        with (
            tc.tile_pool(name="consts", bufs=1) as consts,
            tc.tile_pool(name="stage", bufs=bufs) as stagep,
        ):
            # ---- broadcast the three rate scalars to all partitions ----
            bt = consts.tile([P, 1], f32, tag="bt")
            gt = consts.tile([P, 1], f32, tag="gt")
            st = consts.tile([P, 1], f32, tag="st")
            for dst, src in ((bt, beta), (gt, gamma), (st, sigma)):
                src_ap = src[:]
                bcast = bass.AP(
                    tensor=src_ap.tensor,
                    offset=src_ap.offset,
                    ap=[[0, P], [1, 1]],
                )
                nc.sync.dma_start(out=dst[:, :], in_=bcast)

            # derived per-partition scalars (all prepared on DVE)
            k1t = consts.tile([P, 1], f32, tag="k1")  # c1 = beta/2
            a2t = consts.tile([P, 1], f32, tag="a2")  # 1 - sigma/2
            a3t = consts.tile([P, 1], f32, tag="a3")  # 1 - gamma/2
            c2t = consts.tile([P, 1], f32, tag="c2")  # sigma/2
            m1t = consts.tile([P, 1], f32, tag="m1")  # c1*c2
            nc.vector.tensor_scalar_mul(k1t[:, :], bt[:, :], 0.5)
            nc.vector.tensor_scalar(a2t[:, :], st[:, :], -0.5, 1.0, mult, add)
            nc.vector.tensor_scalar(a3t[:, :], gt[:, :], -0.5, 1.0, mult, add)
            nc.vector.tensor_scalar_mul(c2t[:, :], st[:, :], 0.5)
            nc.vector.tensor_mul(m1t[:, :], k1t[:, :], c2t[:, :])
            a2 = a2t[:, 0:1]
            a3 = a3t[:, 0:1]

            # ---- initial state into block 0, step row 0 ----
            tmp0 = consts.tile([P, 4 * G], f32, tag="init_tmp")
            nc.sync.dma_start(
                out=tmp0[:, :].rearrange("p (c g) -> p c g", c=4),
                in_=init[:, :].rearrange("c (p g) -> p c g", p=P),
            )
            tv = tmp0[:, :].rearrange("p (c g) -> p c g", c=4)  # S,E,I,R

            nk1t = consts.tile([P, 1], f32, tag="nk1")  # -c1
            nc.vector.tensor_scalar_mul(nk1t[:, :], k1t[:, :], -1.0)

            cur = stagep.tile([P, C * seg], f32, tag="stage")
            r = cur[:, :].rearrange("p (c k g) -> p c k g", c=C, k=kk)
            # row 0: s = c1c2*S ; q~ = -c1*I ; e = c1c2*E ; N = s*q~
            nc.vector.tensor_scalar_mul(r[:, 1, 0, :], tv[:, 0, :], m1t[:, 0:1])
            nc.vector.tensor_scalar_mul(r[:, 2, 0, :], tv[:, 2, :], nk1t[:, 0:1])
            nc.vector.tensor_scalar_mul(r[:, 3, 0, :], tv[:, 1, :], m1t[:, 0:1])
            nc.vector.tensor_mul(r[:, 0, 0, :], r[:, 1, 0, :], r[:, 2, 0, :])

            prev_r, prev_k = r, 0
            first = True
            for blk in range(nblk * passes):
                if scratch_out:
                    dma_dst, blk_out = outd, blk % nblk
                else:
                    dma_dst, blk_out = out, (blk % nblk) % out_blks
                if not first and not compute_off:
                    cur = stagep.tile([P, C * seg], f32, tag="stage")
                    r = cur[:, :].rearrange("p (c k g) -> p c k g", c=C, k=kk)
                ks = range(1, kk) if first else range(kk)
                first = False
                for k in ks:
                    if compute_off:
                        break
                    pN = prev_r[:, 0, prev_k, :]
                    pS = prev_r[:, 1, prev_k, :]
                    pQ = prev_r[:, 2, prev_k, :]
                    pE = prev_r[:, 3, prev_k, :]
                    # op1: s' = (q~ + 1) * s
                    nc.vector.scalar_tensor_tensor(
                        r[:, 1, k, :], pQ, 1.0, pS, add, mult
                    )
                    # op2: q~' = (a3*q~) - e
                    nc.vector.scalar_tensor_tensor(
                        r[:, 2, k, :], pQ, a3, pE, mult, subtract
                    )
                    # op3: e' = (a2*e) - N
                    nc.vector.scalar_tensor_tensor(
                        r[:, 3, k, :], pE, a2, pN, mult, subtract
                    )
                    # op4: N' = s' * q~'  (consumed only by next iter's op3)
                    nc.vector.tensor_mul(
                        r[:, 0, k, :], r[:, 1, k, :], r[:, 2, k, :]
                    )
                    prev_r, prev_k = r, k
                # store slots 1..3 (s,e,i) of the whole block: ONE contiguous
                # run, single DMA carrying a single DVE data wait.
                if dma_off:
                    continue
                nc.sync.dma_start(
                    out=dma_dst[:, blk_out, 0 : 3 * seg],
                    in_=cur[:, seg : 4 * seg],
                )

            if chain:
                cht = consts.tile([1, 1], f32, tag="chain")
                nc.sync.dma_start(out=cht[:, :], in_=chain_in[:, :])
                chv = consts.tile([1, 1], f32, tag="chainv")
                last_elem = r[0:1, 3, kk - 1, 0:1]
                nc.vector.tensor_scalar_mul(chv[:, :], last_elem, cht[0:1, 0:1])
                nc.sync.dma_start(out=chain_out[:, :], in_=chv[:, :])

    _legalize_sync(nc)
    return nc


def _legalize_sync(nc):
    # Legalize for walrus' sync-wait limits (2 per compute instruction, 1 per
    # DMA/Pool instruction):
    #  - DMACopy: drop DMA-lane ordering waits (FIFO ring + fixed +16 incs
    #    make them redundant when a data wait is present).
    #  - Drain: keep only the last DMA wait (the final block-store DMA waits
    #    on the compute engine, so DMA completion implies all engines).
    #  - Other instructions: drop sem-ge waits on their OWN engine's sem.
    for bb in nc.m.functions[0].blocks:
        for ins in bb.instructions:
            si = ins.sync_info
            if si is None:
                continue
            ow = si.on_wait
            if not ow or len(ow) < 2:
                continue
            kind = ins.__class__.__name__
            eng = str(ins.engine).rsplit(".", 1)[-1]
            if kind == "InstDMACopy":
                new_w = [
                    w
                    for w in ow
                    if not (
                        w.ant_name.startswith("DMAHW")
                        or w.ant_name.startswith("DMASW")
                    )
                ]
            elif kind == "InstDrain":
                dma_w = [w for w in ow if w.ant_name.startswith("DMA")]
                new_w = dma_w[-1:] if dma_w else ow[-1:]
            else:
                new_w = [
                    w
                    for w in ow
                    if not (
                        w.wait_mode == "sem-ge-imm"
                        and w.ant_name.split("_")[0] == eng
                    )
                ]
            if len(new_w) < len(ow):
                si.on_wait = new_w
                ins.sync_info = si


_nc = None


def kernel(initial, beta, gamma, sigma, t):
    global _nc
    assert int(t) == T
    initial = np.ascontiguousarray(np.asarray(initial, dtype=np.float32))
    beta = np.asarray(beta, dtype=np.float32).reshape(1)
    gamma = np.asarray(gamma, dtype=np.float32).reshape(1)
    sigma = np.asarray(sigma, dtype=np.float32).reshape(1)
    assert initial.shape == (4, B)

    if _nc is None:
        _nc = _build()

    in_maps = []
    for i in range(NCORES):
        shard = np.ascontiguousarray(initial[:, i * BS : (i + 1) * BS])
        in_maps.append(
            {"initial": shard, "beta": beta, "gamma": gamma, "sigma": sigma}
        )

    res = run_bass_kernel_spmd(
        _nc, in_maps, core_ids=list(range(NCORES)), trace=TRACE
    )
    if TRACE and res.exec_time_ns is not None:
        print(f"HW exec time: {res.exec_time_ns} ns")

    # host-side unscramble + unscale; R reconstructed as a cumsum of I
    c1 = np.float64(beta[0]) / 2.0
    c2 = np.float64(sigma[0]) / 2.0
    inv_m1 = np.float32(1.0 / (c1 * c2))
    inv_k1 = np.float32(1.0 / c1)
    half_g = np.float32(np.float64(gamma[0]) / 2.0)

    full = np.empty((T, B, 4), dtype=np.float32)
    for i in range(NCORES):
        # out [P, NBLK, 3*K*G]: slot-major ((s,q~,e), k, g) per block
        arr = res.results[i]["out"].reshape(P, NBLK, 3, K, G)
        # -> (T=blk*K+k, b_local=p*G+g, slot)
        arr = arr.transpose(1, 3, 0, 4, 2).reshape(T, BS, 3)
        dst = full[:, i * BS : (i + 1) * BS, :]
        dst[:, :, 0] = arr[:, :, 0] * inv_m1  # S = s/(c1*c2)
        dst[:, :, 1] = arr[:, :, 2] * inv_m1  # E = e/(c1*c2)
        dst[:, :, 2] = arr[:, :, 1] * (-inv_k1)  # I = -q~/c1
        # R_n = R_0 + (g/2)*sum_{m<n} I_m  (exclusive prefix sum over time)
        dr = np.cumsum(half_g * dst[:, :, 2], axis=0, dtype=np.float32)
        dst[0, :, 3] = initial[3, i * BS : (i + 1) * BS]
        dst[1:, :, 3] = dst[0, :, 3][None, :] + dr[:-1]
    return full.reshape(T * B, 4)


if __name__ == "__main__":
    rng = np.random.default_rng(0)
    ini = rng.random((4, B), dtype=np.float32)
    be, ga, si = (rng.random(1, dtype=np.float32) for _ in range(3))
    outv = kernel(ini, be, ga, si, T)
    print("ran, out shape", outv.shape, outv[:4])
